# revision 7
# baseline (speedup 1.0000x reference)
"""Trainium2 Bass kernel for nn_CoreGroupConstruction (segment_reduce).

Reference: S = Wm @ exp(P) with Wm = row-normalized masked seed weights
([8192, 2048]), P [2048, 2048] edge-independent; loss = bernoulli NLL over
all (edge, node) pairs + degree/size moment losses on row/col sums of S.

Strategy (matches the sharding hint):
 - Host precomputes the tiny edge-independent pieces in f64: theta, P, seed,
   E = exp(P), Wm. O(NC^2) with trivial flops; operands ship in bf16/fp8.
 - Edge dim M=8192 sharded across 8 cores (1024 edges each). Each core runs
   the [1024, 2048] x [2048, 2048] matmul on the tensor engine and reduces
   the pointwise loss -sum log(mask*S + (1-mask)*(1-S)) via the identity
   B = m2*S + b (m2 = 2*mask-1, b = 1-mask): one DVE mul (PSUM read), one
   add, one ACT Ln pass with fused per-partition accumulation.
 - fp8 DoubleRow mode: exact split S = Wm + Wm@F (diag(exp(P)) == 1, F is
   the off-diagonal part, which spans ~one decade so a single power-of-2
   scale keeps it in fp8e4 normal range). The matmul G = Wm@F runs at fp8
   DoubleRow rate (effective K=256 per instruction); the exact diagonal
   part is folded into the host-prepared blend constant C = mask*Wm +
   (1-mask), and the fp8 descale (power of 2) is folded into m2. Then
   B = m2s*G_psum + C identically.
 - Row/col sums of S (size_exp/degree_exp) are exact by associativity:
   sizes = Wm @ rowsum(E), deg = colsum(Wm) @ E - two host f64 matvecs.
 - Host gathers the per-core loss partials in f64, sorts the [2048]/[8192]
   sum vectors, and assembles the final scalar.
"""

import os

import numpy as np
import ml_dtypes

import concourse.bacc as bacc
import concourse.tile as tile
from concourse import mybir
from concourse.bass_utils import run_bass_kernel_spmd

M, NC, K = 8192, 2048, 32
N_CORES = 8
MLOC = M // N_CORES          # 1024 edges per core
P_DIM = 128
ET = MLOC // P_DIM           # 8 edge tiles per core
IC = NC // P_DIM             # 16 contraction chunks (bf16) / 8 double (fp8)
JBLK = 512                   # one f32 PSUM bank
NJ = NC // JBLK              # 4 j-groups

MODE = os.environ.get("BASS_MODE", "fp8dr")   # "bf16" | "fp8dr"

_BF16 = ml_dtypes.bfloat16

_cache = {}


def _build_bass(mode):
    nc = bacc.Bacc("TRN2", target_bir_lowering=False, debug=False)
    bf16 = mybir.dt.bfloat16
    fp8 = mybir.dt.float8e4
    f32 = mybir.dt.float32

    if mode == "bf16":
        eb_d = nc.dram_tensor("eb", [NJ, P_DIM, IC, JBLK], bf16, kind="ExternalInput")
        wm_d = nc.dram_tensor("wm", [ET, P_DIM, IC, P_DIM], bf16, kind="ExternalInput")
    else:
        ic2 = IC // 2
        eb_d = nc.dram_tensor("eb", [NJ, P_DIM, ic2, 2, JBLK], fp8, kind="ExternalInput")
        wm_d = nc.dram_tensor("wm", [ET, P_DIM, ic2, 2, P_DIM], fp8, kind="ExternalInput")
    m2_d = nc.dram_tensor("m2", [ET, P_DIM, NC], bf16, kind="ExternalInput")
    cc_d = nc.dram_tensor("cc", [ET, P_DIM, NC], bf16, kind="ExternalInput")
    loss_d = nc.dram_tensor("loss_pp", [P_DIM, ET * NJ], f32, kind="ExternalOutput")

    with tile.TileContext(nc) as tc:
        with (
            tc.tile_pool(name="const", bufs=1) as cpool,
            tc.tile_pool(name="wmp", bufs=3) as wpool,
            tc.tile_pool(name="mbp", bufs=3) as mbpool,
            tc.tile_pool(name="work", bufs=3) as workpool,
            tc.tile_pool(name="psum", bufs=2, space="PSUM") as pspool,
        ):
            loss_pp = cpool.tile([P_DIM, ET * NJ], f32, tag="loss")

            def load_et(et, w_eng, m_eng, c_eng):
                # spread the streamed per-edge-tile operands across queues
                if mode == "bf16":
                    w = wpool.tile([P_DIM, IC, P_DIM], bf16, tag="wm")
                else:
                    w = wpool.tile([P_DIM, IC // 2, 2, P_DIM], fp8, tag="wm")
                w_eng.dma_start(w[:], wm_d[et])
                m = mbpool.tile([P_DIM, NC], bf16, tag="m2")
                m_eng.dma_start(m[:], m2_d[et])
                c = mbpool.tile([P_DIM, NC], bf16, tag="cc")
                c_eng.dma_start(c[:], cc_d[et])
                return w, m, c

            first = load_et(0, nc.sync, nc.scalar, nc.sync)

            def load_eb(g, eng):
                if mode == "bf16":
                    t = cpool.tile([P_DIM, IC, JBLK], bf16, tag=f"eb{g}")
                else:
                    t = cpool.tile([P_DIM, IC // 2, 2, JBLK], fp8, tag=f"eb{g}")
                eng.dma_start(t[:], eb_d[g])
                return t

            # j-group 0 on the otherwise-idle gpsimd queue, 3 on scalar,
            # 1 on sync (after wm0), 2 on gpsimd second — matches the
            # arrival-order consumption below
            eb_tiles = [None] * NJ
            eb_tiles[0] = load_eb(0, nc.gpsimd)
            eb_tiles[3] = load_eb(3, nc.scalar)
            eb_tiles[1] = load_eb(1, nc.sync)
            eb_tiles[2] = load_eb(2, nc.gpsimd)

            for et in range(ET):
                if et > 0:
                    wm_t, m2_t, cc_t = load_et(
                        et, nc.gpsimd, nc.scalar, nc.sync
                    )
                else:
                    wm_t, m2_t, cc_t = first

                ps = pspool.tile([P_DIM, NC], f32, tag="ps")
                for jc in ([0, 3, 1, 2] if et == 0 else range(NJ)):
                    js = slice(jc * JBLK, (jc + 1) * JBLK)
                    if mode == "bf16":
                        for ic in range(IC):
                            nc.tensor.matmul(
                                ps[:, js],
                                wm_t[:, ic, :],
                                eb_tiles[jc][:, ic, :],
                                start=(ic == 0),
                                stop=(ic == IC - 1),
                            )
                    else:
                        for ic2 in range(IC // 2):
                            nc.tensor.matmul(
                                ps[:, js],
                                wm_t[:, ic2, :, :],
                                eb_tiles[jc][:, ic2, :, :],
                                start=(ic2 == 0),
                                stop=(ic2 == IC // 2 - 1),
                                perf_mode=mybir.MatmulPerfMode.DoubleRow,
                            )
                    # per-bank blend B = m2*S + C (fp8 descale folded into
                    # m2) overlaps the remaining banks' matmuls
                    b_t = workpool.tile([P_DIM, JBLK], f32, tag="B")
                    nc.vector.tensor_mul(b_t[:], ps[:, js], m2_t[:, js])
                    nc.vector.tensor_add(b_t[:], b_t[:], cc_t[:, js])
                    scr = workpool.tile([P_DIM, JBLK], f32, tag="scr")
                    nc.scalar.activation(
                        scr[:], b_t[:], mybir.ActivationFunctionType.Ln,
                        accum_out=loss_pp[:, et * NJ + jc:et * NJ + jc + 1],
                    )

            nc.sync.dma_start(loss_d[:], loss_pp[:])
    nc.compile()
    return nc


def _host_precompute(theta_log, seed_prob, Ic, c2a):
    theta = -np.logaddexp(0.0, -theta_log.astype(np.float64))  # log_sigmoid [K,3]
    A = c2a.astype(np.float64)
    nA = 1.0 - A
    t0, t1, t2 = theta[:, 0], theta[:, 1], theta[:, 2]
    P = (nA * t0) @ nA.T + (A * t1) @ nA.T + (nA * t1) @ A.T + (A * t2) @ A.T
    np.fill_diagonal(P, 0.0)
    sp = seed_prob.astype(np.float64)
    seed = np.exp(sp - sp.max())
    seed /= seed.sum()
    E = np.exp(P)                                # [NC, NC], diag == 1
    Icf = Ic.astype(np.float64)
    rs = Icf @ seed                              # [M]
    Wm = (Icf * seed[None, :]) / rs[:, None]     # [M, NC]
    return E, Wm, Icf


def _make_in_maps(mode, E, Wm, Ic):
    in_maps = []
    if mode == "bf16":
        # eb[jg, p, ic, q] = E[ic*128+p, jg*512+q]
        eb_np = np.ascontiguousarray(
            E.reshape(IC, P_DIM, NJ, JBLK).transpose(2, 1, 0, 3)
        ).astype(_BF16)
    else:
        fp8_np = mybir.dt.np(mybir.dt.float8e4)
        fmax = float(ml_dtypes.finfo(fp8_np).max)
        F = E.copy()
        np.fill_diagonal(F, 0.0)
        sf = 2.0 ** np.floor(np.log2((0.5 * fmax) / F.max()))
        swmax = Wm.max()
        sw = 2.0 ** np.floor(np.log2((0.5 * fmax) / swmax))
        eb_np = np.ascontiguousarray(
            (F * sf).reshape(IC // 2, 2, P_DIM, NJ, JBLK).transpose(3, 2, 0, 1, 4)
        ).astype(fp8_np)
        descale = 1.0 / (sf * sw)

    for c in range(N_CORES):
        sl = slice(c * MLOC, (c + 1) * MLOC)
        Wc = Wm[sl]                              # [1024, 2048]
        mask = Ic[sl].astype(np.float64)
        if mode == "bf16":
            # wm[et, p, ic, el] = Wc[et*128+el, ic*128+p]
            wm_np = np.ascontiguousarray(
                Wc.reshape(ET, P_DIM, IC, P_DIM).transpose(0, 3, 2, 1)
            ).astype(_BF16)
            m2_np = (2.0 * mask - 1.0).reshape(ET, P_DIM, NC).astype(_BF16)
            cc_np = (1.0 - mask).reshape(ET, P_DIM, NC).astype(_BF16)
        else:
            wm_np = np.ascontiguousarray(
                (Wc * sw).reshape(ET, P_DIM, IC // 2, 2, P_DIM).transpose(0, 4, 2, 3, 1)
            ).astype(fp8_np)
            m2_np = ((2.0 * mask - 1.0) * descale).reshape(ET, P_DIM, NC).astype(_BF16)
            cc_np = (mask * Wc + (1.0 - mask)).reshape(ET, P_DIM, NC).astype(_BF16)
        in_maps.append({"eb": eb_np, "wm": wm_np, "m2": m2_np, "cc": cc_np})
    return in_maps


def kernel(theta_log, seed_prob, Ic, c2a):
    assert Ic.shape == (M, NC) and c2a.shape == (NC, K)
    E, Wm, Icf = _host_precompute(theta_log, seed_prob, Ic, c2a)
    in_maps = _make_in_maps(MODE, E, Wm, Ic)

    if MODE not in _cache:
        _cache[MODE] = _build_bass(MODE)
    res = run_bass_kernel_spmd(_cache[MODE], in_maps, core_ids=list(range(N_CORES)))

    loss = -sum(
        r["loss_pp"].astype(np.float64).sum() for r in res.results
    )
    # row/col sums of S, exact by associativity (f64)
    deg = Wm.sum(axis=0) @ E                     # [NC]
    sizes = Wm @ E.sum(axis=1)                   # [M]
    degree_exp = np.sort(deg)[::-1]
    size_exp = np.sort(sizes)[::-1]
    degree_ans = np.sort(Icf.sum(axis=0))[::-1]
    size_ans = np.sort(Icf.sum(axis=1))[::-1]
    degree_loss = np.mean((degree_exp - degree_ans) ** 2)
    size_loss = np.mean((size_exp - size_ans) ** 2)
    return np.float32(loss + degree_loss + size_loss)


# revision 8
# speedup vs baseline: 1.2054x; 1.2054x over previous
"""Trainium2 Bass kernel for nn_CoreGroupConstruction (segment_reduce).

Reference: S = Wm @ exp(P) with Wm = row-normalized masked seed weights
([8192, 2048]), P [2048, 2048] edge-independent; loss = bernoulli NLL over
all (edge, node) pairs + degree/size moment losses on row/col sums of S.

Strategy (matches the sharding hint):
 - Host precomputes the tiny edge-independent pieces in f64: theta, P, seed,
   E = exp(P), Wm. O(NC^2) with trivial flops; operands ship in bf16/fp8.
 - Edge dim M=8192 sharded across 8 cores (1024 edges each). Each core runs
   the [1024, 2048] x [2048, 2048] matmul on the tensor engine and reduces
   the pointwise loss -sum log(mask*S + (1-mask)*(1-S)) via the identity
   B = m2*S + b (m2 = 2*mask-1, b = 1-mask): one DVE mul (PSUM read), one
   add, one ACT Ln pass with fused per-partition accumulation.
 - fp8 DoubleRow mode: exact split S = Wm + Wm@F (diag(exp(P)) == 1, F is
   the off-diagonal part, which spans ~one decade so a single power-of-2
   scale keeps it in fp8e4 normal range). The matmul G = Wm@F runs at fp8
   DoubleRow rate (effective K=256 per instruction); the exact diagonal
   part is folded into the host-prepared blend constant C = mask*Wm +
   (1-mask), and the fp8 descale (power of 2) is folded into m2. Then
   B = m2s*G_psum + C identically.
 - Row/col sums of S (size_exp/degree_exp) are exact by associativity:
   sizes = Wm @ rowsum(E), deg = colsum(Wm) @ E - two host f64 matvecs.
 - Host gathers the per-core loss partials in f64, sorts the [2048]/[8192]
   sum vectors, and assembles the final scalar.
"""

import os

import numpy as np
import ml_dtypes

import concourse.bacc as bacc
import concourse.tile as tile
from concourse import mybir
from concourse.bass_utils import run_bass_kernel_spmd

M, NC, K = 8192, 2048, 32
N_CORES = 8
MLOC = M // N_CORES          # 1024 edges per core
P_DIM = 128
ET = MLOC // P_DIM           # 8 edge tiles per core
IC = NC // P_DIM             # 16 contraction chunks (bf16) / 8 double (fp8)
JBLK = 512                   # one f32 PSUM bank
NJ = NC // JBLK              # 4 j-groups

MODE = os.environ.get("BASS_MODE", "fp8dr")   # "bf16" | "fp8dr"

_BF16 = ml_dtypes.bfloat16

_cache = {}


def _build_bass(mode):
    nc = bacc.Bacc("TRN2", target_bir_lowering=False, debug=False)
    bf16 = mybir.dt.bfloat16
    fp8 = mybir.dt.float8e4
    f32 = mybir.dt.float32

    if mode == "bf16":
        eb_d = nc.dram_tensor("eb", [NJ, P_DIM, IC, JBLK], bf16, kind="ExternalInput")
        wm_d = nc.dram_tensor("wm", [ET, P_DIM, IC, P_DIM], bf16, kind="ExternalInput")
    else:
        ic2 = IC // 2
        eb_d = nc.dram_tensor("eb", [NJ, P_DIM, ic2, 2, JBLK], fp8, kind="ExternalInput")
        wm_d = nc.dram_tensor("wm", [ET, P_DIM, ic2, 2, P_DIM], fp8, kind="ExternalInput")
    m2_d = nc.dram_tensor("m2", [ET, P_DIM, NC], bf16, kind="ExternalInput")
    cc_d = nc.dram_tensor("cc", [ET, P_DIM, NC], bf16, kind="ExternalInput")
    NH = 2                     # loss-blend chunks per edge tile (half tiles)
    HBLK = NC // NH
    loss_d = nc.dram_tensor("loss_pp", [P_DIM, ET * NH], f32, kind="ExternalOutput")

    with tile.TileContext(nc) as tc:
        with (
            tc.tile_pool(name="const", bufs=1) as cpool,
            tc.tile_pool(name="wmp", bufs=3) as wpool,
            tc.tile_pool(name="mbp", bufs=3) as mbpool,
            tc.tile_pool(name="bwork", bufs=6) as bpool,
            tc.tile_pool(name="swork", bufs=3) as spool,
            tc.tile_pool(name="psum", bufs=2, space="PSUM") as pspool,
        ):
            loss_pp = cpool.tile([P_DIM, ET * NH], f32, tag="loss")

            def load_wm(et, eng):
                if mode == "bf16":
                    w = wpool.tile([P_DIM, IC, P_DIM], bf16, tag="wm")
                else:
                    w = wpool.tile([P_DIM, IC // 2, 2, P_DIM], fp8, tag="wm")
                eng.dma_start(w[:], wm_d[et])
                return w

            def load_eb(g, eng):
                if mode == "bf16":
                    t = cpool.tile([P_DIM, IC, JBLK], bf16, tag=f"eb{g}")
                else:
                    t = cpool.tile([P_DIM, IC // 2, 2, JBLK], fp8, tag=f"eb{g}")
                eng.dma_start(t[:], eb_d[g])
                return t

            def load_mc(et):
                m = mbpool.tile([P_DIM, NC], bf16, tag="m2")
                nc.scalar.dma_start(m[:], m2_d[et])
                c = mbpool.tile([P_DIM, NC], bf16, tag="cc")
                nc.sync.dma_start(c[:], cc_d[et])
                return m, c

            # DMA queue plan (per-queue BW ~150 GB/s, three queues):
            #   sync:   wm0, ebj1, cc0, wm1, cc1, wm2, cc2, ...
            #   scalar: ebj2, m2_0, m2_1, ...   (+ the Ln activations)
            #   gpsimd: ebj0, ebj3              (nothing else -> ebj3 early)
            wm_0 = load_wm(0, nc.sync)
            eb_tiles = [None] * NJ
            eb_tiles[0] = load_eb(0, nc.gpsimd)
            eb_tiles[1] = load_eb(1, nc.sync)
            eb_tiles[2] = load_eb(2, nc.scalar)
            eb_tiles[3] = load_eb(3, nc.gpsimd)
            mc_0 = load_mc(0)

            for et in range(ET):
                if et > 0:
                    wm_t = load_wm(et, nc.sync)
                    m2_t, cc_t = load_mc(et)
                else:
                    wm_t, (m2_t, cc_t) = wm_0, mc_0

                ps = pspool.tile([P_DIM, NC], f32, tag="ps")
                for jc in range(NJ):
                    js = slice(jc * JBLK, (jc + 1) * JBLK)
                    if mode == "bf16":
                        for ic in range(IC):
                            nc.tensor.matmul(
                                ps[:, js],
                                wm_t[:, ic, :],
                                eb_tiles[jc][:, ic, :],
                                start=(ic == 0),
                                stop=(ic == IC - 1),
                            )
                    else:
                        for ic2 in range(IC // 2):
                            nc.tensor.matmul(
                                ps[:, js],
                                wm_t[:, ic2, :, :],
                                eb_tiles[jc][:, ic2, :, :],
                                start=(ic2 == 0),
                                stop=(ic2 == IC // 2 - 1),
                                perf_mode=mybir.MatmulPerfMode.DoubleRow,
                            )
                # blend B = m2*S + C (fp8 descale folded into m2) in half
                # tiles: the PSUM-reading mul releases banks for the next
                # edge tile; add/Ln have slack and deep b_t buffering
                for h in range(NH):
                    hs = slice(h * HBLK, (h + 1) * HBLK)
                    b_t = bpool.tile([P_DIM, HBLK], f32, tag="B")
                    nc.vector.tensor_mul(b_t[:], ps[:, hs], m2_t[:, hs])
                    nc.vector.tensor_add(b_t[:], b_t[:], cc_t[:, hs])
                    scr = spool.tile([P_DIM, HBLK], f32, tag="scr")
                    nc.scalar.activation(
                        scr[:], b_t[:], mybir.ActivationFunctionType.Ln,
                        accum_out=loss_pp[:, et * NH + h:et * NH + h + 1],
                    )

            nc.sync.dma_start(loss_d[:], loss_pp[:])
    nc.compile()
    return nc


def _host_precompute(theta_log, seed_prob, Ic, c2a):
    theta = -np.logaddexp(0.0, -theta_log.astype(np.float64))  # log_sigmoid [K,3]
    A = c2a.astype(np.float64)
    nA = 1.0 - A
    t0, t1, t2 = theta[:, 0], theta[:, 1], theta[:, 2]
    P = (nA * t0) @ nA.T + (A * t1) @ nA.T + (nA * t1) @ A.T + (A * t2) @ A.T
    np.fill_diagonal(P, 0.0)
    sp = seed_prob.astype(np.float64)
    seed = np.exp(sp - sp.max())
    seed /= seed.sum()
    E = np.exp(P)                                # [NC, NC], diag == 1
    Icf = Ic.astype(np.float64)
    rs = Icf @ seed                              # [M]
    Wm = (Icf * seed[None, :]) / rs[:, None]     # [M, NC]
    return E, Wm, Icf


def _make_in_maps(mode, E, Wm, Ic):
    in_maps = []
    if mode == "bf16":
        # eb[jg, p, ic, q] = E[ic*128+p, jg*512+q]
        eb_np = np.ascontiguousarray(
            E.reshape(IC, P_DIM, NJ, JBLK).transpose(2, 1, 0, 3)
        ).astype(_BF16)
    else:
        fp8_np = mybir.dt.np(mybir.dt.float8e4)
        fmax = float(ml_dtypes.finfo(fp8_np).max)
        F = E.copy()
        np.fill_diagonal(F, 0.0)
        sf = 2.0 ** np.floor(np.log2((0.5 * fmax) / F.max()))
        swmax = Wm.max()
        sw = 2.0 ** np.floor(np.log2((0.5 * fmax) / swmax))
        eb_np = np.ascontiguousarray(
            (F * sf).reshape(IC // 2, 2, P_DIM, NJ, JBLK).transpose(3, 2, 0, 1, 4)
        ).astype(fp8_np)
        descale = 1.0 / (sf * sw)

    for c in range(N_CORES):
        sl = slice(c * MLOC, (c + 1) * MLOC)
        Wc = Wm[sl]                              # [1024, 2048]
        mask = Ic[sl].astype(np.float64)
        if mode == "bf16":
            # wm[et, p, ic, el] = Wc[et*128+el, ic*128+p]
            wm_np = np.ascontiguousarray(
                Wc.reshape(ET, P_DIM, IC, P_DIM).transpose(0, 3, 2, 1)
            ).astype(_BF16)
            m2_np = (2.0 * mask - 1.0).reshape(ET, P_DIM, NC).astype(_BF16)
            cc_np = (1.0 - mask).reshape(ET, P_DIM, NC).astype(_BF16)
        else:
            wm_np = np.ascontiguousarray(
                (Wc * sw).reshape(ET, P_DIM, IC // 2, 2, P_DIM).transpose(0, 4, 2, 3, 1)
            ).astype(fp8_np)
            m2_np = ((2.0 * mask - 1.0) * descale).reshape(ET, P_DIM, NC).astype(_BF16)
            cc_np = (mask * Wc + (1.0 - mask)).reshape(ET, P_DIM, NC).astype(_BF16)
        in_maps.append({"eb": eb_np, "wm": wm_np, "m2": m2_np, "cc": cc_np})
    return in_maps


def kernel(theta_log, seed_prob, Ic, c2a):
    assert Ic.shape == (M, NC) and c2a.shape == (NC, K)
    E, Wm, Icf = _host_precompute(theta_log, seed_prob, Ic, c2a)
    in_maps = _make_in_maps(MODE, E, Wm, Ic)

    if MODE not in _cache:
        _cache[MODE] = _build_bass(MODE)
    res = run_bass_kernel_spmd(_cache[MODE], in_maps, core_ids=list(range(N_CORES)))

    loss = -sum(
        r["loss_pp"].astype(np.float64).sum() for r in res.results
    )
    # row/col sums of S, exact by associativity (f64)
    deg = Wm.sum(axis=0) @ E                     # [NC]
    sizes = Wm @ E.sum(axis=1)                   # [M]
    degree_exp = np.sort(deg)[::-1]
    size_exp = np.sort(sizes)[::-1]
    degree_ans = np.sort(Icf.sum(axis=0))[::-1]
    size_ans = np.sort(Icf.sum(axis=1))[::-1]
    degree_loss = np.mean((degree_exp - degree_ans) ** 2)
    size_loss = np.mean((size_exp - size_ans) ** 2)
    return np.float32(loss + degree_loss + size_loss)
